# revision 12
# baseline (speedup 1.0000x reference)
"""Trainium2 Bass kernel for per-sample-routed ConvTranspose1d (Dereverb T60
decoder).

Math: for each sample b with routed weight W (Cin=512, K=16), stride 8, pad 8:
    y[t] = A[p, m+1] + A[p+8, m]   where t = 8m + p (p in [0,8), m in [0,3999)),
    A[k, q] = sum_ci W[ci, k] * x[ci, q]        (a 16x512 @ 512x4000 matmul)

Sharding: pure data parallel, B=16 -> 2 samples on each of 8 NeuronCores.
Routing (t60 -> 1 of 41 kernels) is a host-side gather of 32KB per sample.

Design (bf16, DMA-bound):
  - host converts x and the routed W to bf16; W is pre-packed into the exact
    SBUF layout w40[p, c, col] (taps 0..7 at cols 0..7, taps 8..15 at cols
    32..39, zeros elsewhere) so the device load is one clean DMA.
  - x loads as ONE DMA per sample ([128, 4, 4000] bf16, 8KB runs per
    (partition, chunk) row) on the sync ring, which x owns exclusively --
    measured: sharing that queue with w, splitting across rings, or
    host-sequential HBM layout are all slower.
  - 8 j-tiles of 500 output columns; each tile's matmul computes A for 501
    q-columns (500*j .. 500*j+500 inclusive), so the shifted pair-add
    z[p, m] = ps[p, m+1-j0] + ps[32+p, m-j0] is self-contained per tile
    (both operands from the same PSUM tile, no cross-tile boundary work).
  - bf16 matmul runs at 1 cycle/row (fp32 is 4); PSUM accumulation is fp32.
  - engine ops can read only ONE PSUM operand, so the pair-add is staged:
    ACT (scalar) copies the hi taps psum->z, DVE adds the shifted lo taps.
  - no PE transposes / staging copies: z[8, 3999] bf16 is DMA'd out
    contiguously and the final p-interleave y[8m+p] = z[p, m] is a cheap
    host-side reshape. This removes ~4000 32-byte DMA packets per sample
    that made the baseline DMA-bound at 275 GB/s.
Steady state: DMA ~97% busy (~8.4 MB/rep at ~400 GB/s), Tensor ~85% duty
(17.9 us/rep = bf16 floor), slope ~21.1 us/rep vs 20.2 us measured pure-DMA
floor. The residual gap is power throttling: with compute running, the core
spends ~30% of time at a 0.5 utilization cap (dma-only runs show zero
throttle), so less engine work per byte is the only lever left.
fp8 x fails the 2e-2 gate (measured 2.7e-2 even with bf16 W); shipping raw
A taps to the host loses (keeps tensor power, adds ~0.9 us of DMA).
"""
import numpy as np
import ml_dtypes

import concourse.bass as bass
import concourse.tile as tile
from concourse import bacc, mybir
from concourse.bass_utils import run_bass_kernel_spmd

B, CIN, L, KSZ = 16, 512, 4000, 16
LOUT = (L - 1) * 8 - 2 * 8 + KSZ  # 31992
NCORES = 8
PER = B // NCORES                 # 2 samples per core
NCHUNK = CIN // 128               # 4
JW = 500                          # j-tile output width
NJ = 8
MV = L - 1                        # 3999 valid output m positions
F32 = mybir.dt.float32
BF16 = mybir.dt.bfloat16

_CACHE = {}


def _build(reps=1, mode="full", xbufs=4, pabufs=8, zbufs=3, wbufs=3):
    # x loads as ONE DMA per sample: [128, 4, 4000] bf16, 8KB contiguous
    # runs per (partition, chunk) row. Bigger packets -> ~420 GB/s vs
    # ~326 GB/s with 2KB runs, and 1 issue instead of 4 on the Sync ring
    # (each issue measured ~2us).
    nc = bacc.Bacc("TRN2", target_bir_lowering=False, debug=False,
                   num_devices=NCORES)
    x = nc.dram_tensor("x", [PER, CIN, L], BF16, kind="ExternalInput").ap()
    w = nc.dram_tensor("w", [PER, 128, NCHUNK * 40], BF16,
                       kind="ExternalInput").ap()
    y = nc.dram_tensor("y", [PER, 8, MV], BF16, kind="ExternalOutput").ap()

    with tile.TileContext(nc) as tc:
        with tc.tile_pool(name="xp", bufs=xbufs) as xp, \
             tc.tile_pool(name="wp", bufs=wbufs) as wp, \
             tc.tile_pool(name="zp", bufs=zbufs) as zp, \
             tc.tile_pool(name="pa", bufs=pabufs, space="PSUM") as pa:

            for rep in range(reps):
                for s in range(PER):
                    w40 = wp.tile([128, NCHUNK, 40], BF16, tag="w40")
                    nc.scalar.dma_start(
                        w40[:], w[s].rearrange("p (c k) -> p c k", c=NCHUNK))

                    z = zp.tile([8, L], BF16, tag="z")
                    xt = xp.tile([128, NCHUNK, L], BF16, tag="xt")
                    nc.sync.dma_start(
                        xt[:], x[s].rearrange("(c p) l -> p c l", p=128))
                    for j in range(NJ):
                        j0 = JW * j
                        if mode == "dmaonly":
                            continue
                        nq = min(JW + 1, L - j0)   # 501, last tile 500
                        ps = pa.tile([40, JW + 1], F32, tag="pa")
                        for c in range(NCHUNK):
                            nc.tensor.matmul(
                                ps[:, 0:nq], w40[:, c, :],
                                xt[:, c, j0: j0 + nq],
                                start=(c == 0), stop=(c == NCHUNK - 1))
                        nm = min(JW, MV - j0)      # 500, last tile 499
                        # TT can read only one PSUM operand: stage the hi
                        # taps into z on ACT, then add the shifted lo
                        # taps on Vector (one PSUM input each).
                        nc.scalar.copy(
                            z[0:8, j0: j0 + nm], ps[32:40, 0: nm])
                        nc.vector.tensor_tensor(
                            z[0:8, j0: j0 + nm],
                            z[0:8, j0: j0 + nm],
                            ps[0:8, 1: 1 + nm],
                            mybir.AluOpType.add)

                    if mode == "dmaonly":
                        zd = zp.tile([8, L], BF16, tag="zd")
                        nc.vector.memset(zd[:], 0.0)
                        nc.scalar.dma_start(y[s], zd[0:8, 0:MV])
                    else:
                        nc.scalar.dma_start(y[s], z[0:8, 0:MV])

    nc.compile()
    return nc


def _route(t60s):
    idx = np.round(t60s.astype(np.float32) * np.float32(100.0))
    return np.tile(idx.astype(np.int32), 2) - 10  # (B,)


def get_nc(reps=1, f32r=False, mode="full"):
    key = (reps, mode)
    if key not in _CACHE:
        _CACHE[key] = _build(reps=reps, mode=mode)
    return _CACHE[key]


def make_in_maps(input, t60s, kernel_weight):
    idx = _route(np.asarray(t60s))
    wg = np.asarray(kernel_weight)[idx, :, 0, :]      # (B, Cin, K) fp32
    # pack into the SBUF w40 layout: [p, c, col] with taps 0..7 at cols 0..7
    # and taps 8..15 at cols 32..39 (base partitions {0,32} for engine ops).
    w40 = np.zeros((B, 128, NCHUNK, 40), dtype=ml_dtypes.bfloat16)
    wr = wg.reshape(B, NCHUNK, 128, KSZ)              # ci = c*128 + p
    w40[:, :, :, 0:8] = wr.transpose(0, 2, 1, 3)[:, :, :, 0:8]
    w40[:, :, :, 32:40] = wr.transpose(0, 2, 1, 3)[:, :, :, 8:16]
    xin = np.asarray(input, dtype=np.float32).astype(ml_dtypes.bfloat16)
    in_maps = []
    for c in range(NCORES):
        sl = slice(PER * c, PER * (c + 1))
        in_maps.append({
            "x": np.ascontiguousarray(xin[sl]),
            "w": np.ascontiguousarray(
                w40[sl].reshape(PER, 128, NCHUNK * 40)),
        })
    return in_maps


def _run(input, t60s, kernel_weight, trace=False):
    nc = get_nc()
    in_maps = make_in_maps(input, t60s, kernel_weight)
    res = run_bass_kernel_spmd(nc, in_maps, core_ids=list(range(NCORES)),
                               trace=trace)
    out = np.empty((B, 1, LOUT), dtype=np.float32)
    for c in range(NCORES):
        yr = res.results[c]["y"]                      # (PER, 8, MV) bf16
        for s in range(PER):
            # y[8m+p] = yr[s, p, m]
            out[PER * c + s, 0, :] = np.ascontiguousarray(
                yr[s].T).reshape(-1)[:LOUT]
    return out, res


def kernel(input, t60s, kernel_weight):
    out, _ = _run(input, t60s, kernel_weight, trace=False)
    return out
